# revision 22
# baseline (speedup 1.0000x reference)
"""MoE (top-2 of 8 experts, SwiGLU) Trainium2 kernel, 8-core SPMD.

Strategy
--------
NEFF-A (router, token-parallel): each core takes a 2048-token slice of
x^T, computes logits = x @ router_w via PE matmuls (fp32), softmax +
top-2 + renormalized combine coefficients on DVE/ACT.  Outputs per core:
probs [2048,8], coef [2048,8], rowmax m1 [2048], sumexp s [2048].

Host glue: per-expert token index lists from coef (compaction), padded
to capacity C; aux losses (lb/z) reduced exactly in fp64 from device
probs / m1 / s.

NEFF-B (experts, expert-parallel): core e holds expert e's weights in
SBUF, gathers its routed tokens with indirect DMA from a full copy of x,
PE-transposes them to [H, tok] layout, then
  gT = Wg^T xg^T ; uT = Wu^T xg^T ; hT = silu(gT) * uT ; yT = Wd^T hT
with tokens as the 512-wide moving dimension, scales yT by the
renormalized router weight (0 for padding), and writes compact
yT [768, C].  Host scatters: out[idx_e] += yT_e.T.
"""

import math
import os
import sys
import types

import numpy as np

for _p in ("/opt/trn_rl_repo",):
    if os.path.isdir(_p) and _p not in sys.path:
        sys.path.append(_p)

import concourse.bass as bass
import concourse.mybir as mybir
from concourse.tile import TileContext
from concourse.vector_clock import ScopedClock
from concourse.masks import make_identity
from concourse import bass_utils

# ----------------------------------------------------------------------------
# problem constants (hardcoded per spec)
B, S, H, E, I_DIM, TOPK = 8, 2048, 768, 8, 1536, 2
T = B * S          # 16384 tokens
N_CORES = 8
TPC = T // N_CORES  # 2048 tokens per core for the router
P = 128
HT = H // P         # 6 h-tiles
IT = I_DIM // P     # 12 i-tiles

F32 = mybir.dt.float32
F32R = mybir.dt.float32r
BF16 = mybir.dt.bfloat16
I32 = mybir.dt.int32

# matmul dtype for the expert MLP: "bf16" | "f32r" | "f32"
MM_DTYPE = os.environ.get("MOE_MM_DTYPE", "f32r")
CHUNK = 512

AUX_LOSS_COEF = 1e-3
Z_LOSS_COEF = 1e-3


# ----------------------------------------------------------------------------
# toolchain workaround: this walrus build only accepts ONE sync-wait per CTRL
# instruction; Tile's end-of-context drain carries one wait per live proc.
# Split them across multiple drains.
def _split_all_waits(nc):
    """Move extra sync-waits (beyond 1 per instruction) onto injected NOPs on
    the same engine, immediately before the instruction.  Semantically
    identical: the engine blocks on each wait in sequence."""
    cur = nc.cur_bb.bb
    for f in nc.m.functions:
        for bb in f.blocks:
            insts = list(bb.instructions)
            out = []
            changed = False
            for inst in insts:
                si = inst.sync_info
                waits = list(si.on_wait) if si and si.on_wait else []
                if len(waits) > 1 and inst.engine in nc.engines:
                    changed = True
                    for w in waits[:-1]:
                        nop = nc.engines[inst.engine].nop(nofuse=True)
                        cur.instructions.pop()
                        nsi = nop.ins.sync_info
                        if nsi is None:
                            nop.ins.sync_info = mybir.SyncInfo(
                                on_wait=[w], on_update=[]
                            )
                        else:
                            nsi.on_wait = [w]
                        out.append(nop.ins)
                    si.on_wait = waits[-1:]
                out.append(inst)
            if changed:
                bb.instructions = out


class _TC(TileContext):
    def _drain_and_barrier(self, tick_clock, wait_clock):
        drain_inst = self.nc.sync.drain()
        wait_clock.add_sem_waits(
            drain_inst.ins, ScopedClock({None: tick_clock.global_clock})
        )
        self.nc.all_engine_barrier()
        popped = self.nc._tile_sem_poison_stack.pop()
        assert popped is self._sem_poison
        self.nc.clear_and_free_semaphores(list(self.sems.allocated().values()))
        self.nc.all_engine_barrier()
        _split_all_waits(self.nc)


def _install_ntff_hook():
    """Make trace=True usable under axon when antenv.axon_hooks is absent."""
    try:
        import antenv  # noqa: F401
    except ImportError:
        return
    if "antenv.axon_hooks" in sys.modules:
        return
    mod = types.ModuleType("antenv.axon_hooks")
    mod._hook = None

    def set_axon_ntff_profile_hook(h):
        mod._hook = h

    def get_axon_ntff_profile_hook():
        return mod._hook

    mod.set_axon_ntff_profile_hook = set_axon_ntff_profile_hook
    mod.get_axon_ntff_profile_hook = get_axon_ntff_profile_hook
    sys.modules["antenv.axon_hooks"] = mod
    import antenv as _a

    _a.axon_hooks = mod
    try:
        from trn_agent_boot.trn_boot import _ntff_profile_via_ctypes

        hook = _ntff_profile_via_ctypes("/opt/axon/libaxon_pjrt.so")
        if hook is not None:
            set_axon_ntff_profile_hook(hook)
    except Exception:
        pass
    # artifact upload isn't available (and isn't wanted) in this container
    bass_utils.upload_artifacts = lambda tmpdir: tmpdir


_install_ntff_hook()


# ----------------------------------------------------------------------------
def build_router_nc():
    """Token-parallel router NEFF.  Inputs: xT [H, TPC] (this core's token
    slice, transposed on host), rw [H, E].  Outputs (device layout,
    token t = tt*128 + p): probs/coef [P, NTT*E], m1/s [P, NTT]."""
    nc = bass.Bass(trn_type="TRN2")
    NTT = TPC // P  # 16 token tiles
    xT = nc.dram_tensor("xT", [H, TPC], F32, kind="ExternalInput")
    rw = nc.dram_tensor("rw", [H, E], F32, kind="ExternalInput")
    probs_o = nc.dram_tensor("probs", [P, NTT * E], F32, kind="ExternalOutput")
    coef_o = nc.dram_tensor("coef", [P, NTT * E], F32, kind="ExternalOutput")
    m1_o = nc.dram_tensor("m1", [P, NTT], F32, kind="ExternalOutput")
    s_o = nc.dram_tensor("s", [P, NTT], F32, kind="ExternalOutput")

    NCH = 4  # input-token chunks for DMA/MM pipelining
    CW = TPC // NCH  # 512

    with _TC(nc) as tc:
        with (
            tc.tile_pool(name="big", bufs=1) as big,
            tc.tile_pool(name="work", bufs=1) as work,
            tc.tile_pool(name="lps", bufs=4, space="PSUM") as lps,
            tc.tile_pool(name="tps", bufs=4, space="PSUM") as tps,
        ):
            ident = work.tile([P, P], F32)
            make_identity(nc, ident[:])
            rw_sb = work.tile([P, HT * E], F32)
            nc.sync.dma_start(
                out=rw_sb[:].rearrange("p (h e) -> p h e", e=E),
                in_=rw.rearrange("(h p) e -> p h e", p=P),
            )
            xT_sb = big.tile([P, HT * TPC], F32)
            xTv = xT_sb[:].rearrange("p (h t) -> p h t", t=TPC)
            for c in range(NCH):
                nc.sync.dma_start(
                    out=xTv[:, :, c * CW : (c + 1) * CW],
                    in_=xT.rearrange("(h p) t -> p h t", p=P)[
                        :, :, c * CW : (c + 1) * CW
                    ],
                )

            # logitsT [8, tok] accumulated over h with rw stationary
            lT_sb = work.tile([8, TPC], F32)
            for c in range(NCH):
                pl = lps.tile([8, CW], F32, tag="lpsum")
                for h in range(HT):
                    nc.tensor.matmul(
                        out=pl[:],
                        lhsT=rw_sb[:, h * E : (h + 1) * E],
                        rhs=xT_sb[:, h * TPC + c * CW : h * TPC + (c + 1) * CW],
                        start=(h == 0),
                        stop=(h == HT - 1),
                    )
                nc.vector.tensor_copy(out=lT_sb[:, c * CW : (c + 1) * CW], in_=pl[:])

            # transpose back to [tok, E] tiles
            logits_sb = work.tile([P, NTT * E], F32)
            for tt in range(NTT):
                pt = tps.tile([P, E], F32, tag="tp")
                nc.tensor.transpose(
                    out=pt[:],
                    in_=lT_sb[:, tt * P : (tt + 1) * P],
                    identity=ident[:8, :8],
                )
                nc.vector.tensor_copy(
                    out=logits_sb[:, tt * E : (tt + 1) * E], in_=pt[:]
                )

            lg = logits_sb[:].rearrange("p (t e) -> p t e", e=E)
            m1 = work.tile([P, NTT], F32)
            nc.vector.reduce_max(out=m1[:], in_=lg, axis=mybir.AxisListType.X)
            m1b = m1[:].to_broadcast([P, NTT, E])

            ex = work.tile([P, NTT * E], F32)
            exg = ex[:].rearrange("p (t e) -> p t e", e=E)
            nc.vector.tensor_sub(out=exg, in0=lg, in1=m1b)
            nc.scalar.activation(
                out=ex[:], in_=ex[:], func=mybir.ActivationFunctionType.Exp
            )

            s = work.tile([P, NTT], F32)
            nc.vector.reduce_sum(out=s[:], in_=exg, axis=mybir.AxisListType.X)
            rs = work.tile([P, NTT], F32)
            nc.vector.reciprocal(out=rs[:], in_=s[:])
            rsb = rs[:].to_broadcast([P, NTT, E])

            probs = work.tile([P, NTT * E], F32)
            pg = probs[:].rearrange("p (t e) -> p t e", e=E)
            nc.vector.tensor_mul(out=pg, in0=exg, in1=rsb)

            # top-2 + renormalized coefficients
            m1p = work.tile([P, NTT], F32)
            nc.vector.reduce_max(out=m1p[:], in_=pg, axis=mybir.AxisListType.X)
            m1pb = m1p[:].to_broadcast([P, NTT, E])

            lt = work.tile([P, NTT * E], F32)
            ltg = lt[:].rearrange("p (t e) -> p t e", e=E)
            nc.vector.tensor_tensor(
                out=ltg, in0=pg, in1=m1pb, op=mybir.AluOpType.is_lt
            )
            p2 = work.tile([P, NTT * E], F32)
            p2g = p2[:].rearrange("p (t e) -> p t e", e=E)
            nc.vector.tensor_mul(out=p2g, in0=pg, in1=ltg)

            m2p = work.tile([P, NTT], F32)
            nc.vector.reduce_max(out=m2p[:], in_=p2g, axis=mybir.AxisListType.X)
            m2pb = m2p[:].to_broadcast([P, NTT, E])

            ge = work.tile([P, NTT * E], F32)
            geg = ge[:].rearrange("p (t e) -> p t e", e=E)
            nc.vector.tensor_tensor(
                out=geg, in0=pg, in1=m2pb, op=mybir.AluOpType.is_ge
            )

            den = work.tile([P, NTT], F32)
            nc.vector.tensor_add(out=den[:], in0=m1p[:], in1=m2p[:])
            rden = work.tile([P, NTT], F32)
            nc.vector.reciprocal(out=rden[:], in_=den[:])
            rdenb = rden[:].to_broadcast([P, NTT, E])

            coef = work.tile([P, NTT * E], F32)
            cg = coef[:].rearrange("p (t e) -> p t e", e=E)
            nc.vector.tensor_mul(out=cg, in0=pg, in1=geg)
            nc.vector.tensor_mul(out=cg, in0=cg, in1=rdenb)

            # outputs in device layout; host reorders
            nc.sync.dma_start(out=probs_o[:, :], in_=probs[:])
            nc.sync.dma_start(out=coef_o[:, :], in_=coef[:])
            nc.sync.dma_start(out=m1_o[:, :], in_=m1[:])
            nc.sync.dma_start(out=s_o[:, :], in_=s[:])
    return nc


def _dev_to_tok(a, ncols=None):
    """[P, NTT*E] device layout -> [TPC, E] token-major (or [P,NTT] -> [TPC])."""
    if ncols is None:
        return np.ascontiguousarray(a.T).reshape(-1)
    p, te = a.shape
    return a.reshape(p, te // ncols, ncols).transpose(1, 0, 2).reshape(-1, ncols)


# ----------------------------------------------------------------------------
def build_expert_nc(C: int, mm_dtype: str):
    """Expert-parallel MLP NEFF for capacity C (multiple of 128).

    Inputs per core: x [T, H] (full), idx [C] int32, w [C] f32,
    wg [H, I], wu [H, I], wd [I, H] (this core's expert).
    Output: yT [H, C] f32 — w-scaled expert output, transposed.
    """
    md = {"bf16": BF16, "f32r": F32R, "f32": F32}[mm_dtype]
    # f32r is bit-identical to f32: declare weight DRAM tensors as f32r so
    # the (fast) HWDGE path loads them without a SWDGE "cast".
    wdt = md if md in (F32, F32R) else F32
    nc = bass.Bass(trn_type="TRN2")
    x = nc.dram_tensor("x", [T, H], F32, kind="ExternalInput")
    idx = nc.dram_tensor("idx", [C, 1], I32, kind="ExternalInput")
    wvec = nc.dram_tensor("w", [C], F32, kind="ExternalInput")
    wg = nc.dram_tensor("wg", [H, I_DIM], wdt, kind="ExternalInput")
    wu = nc.dram_tensor("wu", [H, I_DIM], wdt, kind="ExternalInput")
    wd = nc.dram_tensor("wd", [I_DIM, H], wdt, kind="ExternalInput")
    yT_o = nc.dram_tensor("yT", [H, C], F32, kind="ExternalOutput")

    # chunk widths
    chunks = []
    c0 = 0
    while c0 < C:
        w_ = min(CHUNK, C - c0)
        chunks.append((c0, w_))
        c0 += w_

    with _TC(nc) as tc:
        with (
            tc.tile_pool(name="const", bufs=1) as const,
            tc.tile_pool(name="wpool", bufs=1) as wpool,
            tc.tile_pool(name="xg", bufs=5) as xgp,
            tc.tile_pool(name="xt", bufs=2) as xtp,
            tc.tile_pool(name="hb", bufs=1) as hbp,
            tc.tile_pool(name="sg", bufs=2) as sgp,
            tc.tile_pool(name="yb", bufs=1) as ybp,
            tc.tile_pool(name="tps", bufs=2, space="PSUM") as tps,
            tc.tile_pool(name="gps", bufs=2, space="PSUM") as gps,
            tc.tile_pool(name="ups", bufs=2, space="PSUM") as ups,
            tc.tile_pool(name="yps", bufs=2, space="PSUM") as yps,
        ):
            ident = const.tile([P, P], F32)
            make_identity(nc, ident[:])

            # index / weight vectors
            idx_sb = const.tile([P, C // P], I32)
            nc.sync.dma_start(
                out=idx_sb[:].rearrange("p (a u) -> p a u", u=1),
                in_=idx.rearrange("(a p) u -> p a u", p=P),
            )

            # expert weights, resident in SBUF
            def load_w(dram, n_in_tiles, n_out):
                tl = []
                for h in range(n_in_tiles):
                    t_ = wpool.tile([P, n_out], md, tag=f"w{dram.name}{h}", name=f"w{dram.name}{h}")
                    if md == dram.dtype:
                        nc.sync.dma_start(
                            out=t_[:], in_=dram[h * P : (h + 1) * P, :]
                        )
                    else:
                        nc.gpsimd.dma_start(
                            out=t_[:], in_=dram[h * P : (h + 1) * P, :]
                        )
                    tl.append(t_)
                return tl

            wg_sb = load_w(wg, HT, I_DIM)
            wu_sb = load_w(wu, HT, I_DIM)
            wd_sb = load_w(wd, IT, H)

            ones_sb = const.tile([1, P], F32)
            nc.vector.memset(ones_sb[:], 1.0)

            TB = int(os.environ.get("MOE_TB", "3"))

            def emit_gather_transpose(c0, W):
                """Fill one xgT tile [128, (h, W)] for tokens [c0, c0+W)."""
                ntt = W // P
                xt_one = xtp.tile([P, HT * W], md, tag="xt", name=f"xt_{c0}")
                xt_v = xt_one[:].rearrange("p (h w) -> p h w", w=W)
                for j in range(ntt):
                    xg = xgp.tile([P, H], F32, tag="xg", name=f"xg_{c0}_{j}")
                    nc.gpsimd.indirect_dma_start(
                        out=xg[:],
                        out_offset=None,
                        in_=x[:, :],
                        in_offset=bass.IndirectOffsetOnAxis(
                            ap=idx_sb[:, c0 // P + j : c0 // P + j + 1], axis=0
                        ),
                    )
                    # TB transposes per PSUM tile, one batched copy per TB
                    for g in range(HT // TB):
                        pt = tps.tile([P, TB * P], F32, tag="tp", name=f"tp{g}_{c0}_{j}")
                        for k in range(TB):
                            h = TB * g + k
                            nc.tensor.matmul(
                                out=pt[:, k * P : (k + 1) * P],
                                lhsT=xg[:, h * P : (h + 1) * P],
                                rhs=ident[:],
                                is_transpose=True,
                                start=True,
                                stop=True,
                                skip_group_check=True,
                            )
                        nc.vector.tensor_copy(
                            out=xt_v[:, TB * g : TB * g + TB, j * P : (j + 1) * P],
                            in_=pt[:].rearrange("p (k q) -> p k q", q=P),
                        )
                return xt_one

            def emit_gemm_phase(c0, W, xt_one):
                # broadcast w row across partitions via PE
                w_sl = ybp.tile([1, W], F32, tag="wsl", name=f"wsl_{c0}", bufs=2)
                nc.sync.dma_start(out=w_sl[:], in_=wvec[None, c0 : c0 + W])
                wps = tps.tile([P, W], F32, tag="tp", name=f"wps_{c0}")
                nc.tensor.matmul(
                    out=wps[:],
                    lhsT=ones_sb[:],
                    rhs=w_sl[:],
                    start=True,
                    stop=True,
                )
                wbc = ybp.tile([P, W], F32, tag="wbc_sb", name=f"wbc_{c0}", bufs=2)
                nc.vector.tensor_copy(out=wbc[:], in_=wps[:])

                # gate/up + swiglu -> hT tiles [128i, W]
                h_tiles = []
                for i in range(IT):
                    pg_ = gps.tile([P, W], F32, tag="gp")
                    pu_ = ups.tile([P, W], F32, tag="up")
                    for h in range(HT):
                        nc.tensor.matmul(
                            out=pg_[:],
                            lhsT=wg_sb[h][:, i * P : (i + 1) * P],
                            rhs=xt_one[:, h * W : (h + 1) * W],
                            start=(h == 0),
                            stop=(h == HT - 1),
                        )
                    for h in range(HT):
                        nc.tensor.matmul(
                            out=pu_[:],
                            lhsT=wu_sb[h][:, i * P : (i + 1) * P],
                            rhs=xt_one[:, h * W : (h + 1) * W],
                            start=(h == 0),
                            stop=(h == HT - 1),
                        )
                    sg_ = sgp.tile([P, W], F32, tag="sg")
                    nc.scalar.activation(
                        out=sg_[:], in_=pg_[:], func=mybir.ActivationFunctionType.Silu
                    )
                    ht = hbp.tile([P, W], md, tag=f"h{i}", name=f"h{i}_{c0}")
                    nc.vector.tensor_mul(out=ht[:], in0=sg_[:], in1=pu_[:])
                    h_tiles.append(ht)

                # down projection + scale + store
                y_sb = ybp.tile([P, HT * W], F32, tag="y", name=f"y_{c0}")
                for o in range(HT):
                    py = yps.tile([P, W], F32, tag="yp", name=f"yp{o}_{c0}")
                    for i in range(IT):
                        nc.tensor.matmul(
                            out=py[:],
                            lhsT=wd_sb[i][:, o * P : (o + 1) * P],
                            rhs=h_tiles[i][:],
                            start=(i == 0),
                            stop=(i == IT - 1),
                        )
                    nc.vector.tensor_mul(
                        out=y_sb[:, o * W : (o + 1) * W], in0=py[:], in1=wbc[:]
                    )
                nc.sync.dma_start(
                    out=yT_o.rearrange("(o p) c -> p o c", p=P)[:, :, c0 : c0 + W],
                    in_=y_sb[:].rearrange("p (o c) -> p o c", c=W),
                )

            # software pipeline: chunk k+1's gather/transpose is emitted before
            # chunk k's GEMM phase, so its PSUM->SBUF casts overlap GEMM MMs.
            pend = None
            for c0, W in chunks:
                xt_one = emit_gather_transpose(c0, W)
                if pend is not None:
                    emit_gemm_phase(*pend)
                pend = (c0, W, xt_one)
            emit_gemm_phase(*pend)
    return nc


# ----------------------------------------------------------------------------
_NC_CACHE = {}


def _get_router_nc():
    if "router" not in _NC_CACHE:
        _NC_CACHE["router"] = build_router_nc()
    return _NC_CACHE["router"]


def _get_expert_nc(C, mm_dtype):
    key = ("expert", C, mm_dtype)
    if key not in _NC_CACHE:
        _NC_CACHE[key] = build_expert_nc(C, mm_dtype)
    return _NC_CACHE[key]


def _run(nc, in_maps, **kw):
    return bass_utils.run_bass_kernel_spmd(
        nc, in_maps, core_ids=list(range(N_CORES)), **kw
    )


def kernel(hidden_states, router_w, w_gate, w_up, w_down, _profile=None):
    x = np.ascontiguousarray(np.asarray(hidden_states, np.float32)).reshape(T, H)
    rw = np.ascontiguousarray(np.asarray(router_w, np.float32))
    wg = np.ascontiguousarray(np.asarray(w_gate, np.float32))
    wu = np.ascontiguousarray(np.asarray(w_up, np.float32))
    wd = np.ascontiguousarray(np.asarray(w_down, np.float32))

    prof = _profile if _profile is not None else {}

    # ---- NEFF-A: router ----
    xT = np.ascontiguousarray(x.T)  # [H, T]
    nc_a = _get_router_nc()
    in_maps = [
        {"xT": np.ascontiguousarray(xT[:, c * TPC : (c + 1) * TPC]), "rw": rw}
        for c in range(N_CORES)
    ]
    res_a = _run(nc_a, in_maps, **prof.get("a_kw", {}))
    prof["res_a"] = res_a

    probs = np.concatenate([_dev_to_tok(r["probs"], E) for r in res_a.results], 0)
    coef = np.concatenate([_dev_to_tok(r["coef"], E) for r in res_a.results], 0)
    m1 = np.concatenate([_dev_to_tok(r["m1"]) for r in res_a.results], 0)
    sums = np.concatenate([_dev_to_tok(r["s"]) for r in res_a.results], 0)

    # ---- host: routing compaction + losses (exact, fp64) ----
    mask = coef > 0.0
    counts = mask.sum(0)
    C = max(int(math.ceil(counts.max() / P) * P), P)
    idx_arr = np.zeros((N_CORES, C, 1), np.int32)
    w_arr = np.zeros((N_CORES, C), np.float32)
    idx_list = []
    for e in range(E):
        ie = np.nonzero(mask[:, e])[0].astype(np.int32)
        idx_list.append(ie)
        idx_arr[e, : len(ie), 0] = ie
        w_arr[e, : len(ie)] = coef[ie, e]

    Pm = probs.astype(np.float64).mean(0)
    f = mask.astype(np.float64).mean(0) / TOPK
    lb_loss = np.float32(AUX_LOSS_COEF * E * np.sum(f * Pm))
    lse = np.log(sums.astype(np.float64)) + m1.astype(np.float64)
    z_loss = np.float32(Z_LOSS_COEF * np.mean(lse**2))

    # ---- NEFF-B: experts ----
    nc_b = _get_expert_nc(C, MM_DTYPE)
    in_maps = [
        {
            "x": x,
            "idx": idx_arr[e],
            "w": w_arr[e],
            "wg": np.ascontiguousarray(wg[e]),
            "wu": np.ascontiguousarray(wu[e]),
            "wd": np.ascontiguousarray(wd[e]),
        }
        for e in range(E)
    ]
    res_b = _run(nc_b, in_maps, **prof.get("b_kw", {}))
    prof["res_b"] = res_b

    out = np.zeros((T, H), np.float32)
    for e in range(E):
        ie = idx_list[e]
        out[ie] += res_b.results[e]["yT"].T[: len(ie)]

    return out.reshape(B, S, H), lb_loss, z_loss


# revision 23
# speedup vs baseline: 1.0257x; 1.0257x over previous
"""MoE (top-2 of 8 experts, SwiGLU) Trainium2 kernel, 8-core SPMD.

Strategy
--------
NEFF-A (router, token-parallel): each core takes a 2048-token slice of
x^T, computes logits = x @ router_w via PE matmuls (fp32), softmax +
top-2 + renormalized combine coefficients on DVE/ACT.  Outputs per core:
probs [2048,8], coef [2048,8], rowmax m1 [2048], sumexp s [2048].

Host glue: per-expert token index lists from coef (compaction), padded
to capacity C; aux losses (lb/z) reduced exactly in fp64 from device
probs / m1 / s.

NEFF-B (experts, expert-parallel): core e holds expert e's weights in
SBUF, gathers its routed tokens with indirect DMA from a full copy of x,
PE-transposes them to [H, tok] layout, then
  gT = Wg^T xg^T ; uT = Wu^T xg^T ; hT = silu(gT) * uT ; yT = Wd^T hT
with tokens as the 512-wide moving dimension, scales yT by the
renormalized router weight (0 for padding), and writes compact
yT [768, C].  Host scatters: out[idx_e] += yT_e.T.
"""

import math
import os
import sys
import types

import numpy as np

for _p in ("/opt/trn_rl_repo",):
    if os.path.isdir(_p) and _p not in sys.path:
        sys.path.append(_p)

import concourse.bass as bass
import concourse.mybir as mybir
from concourse.tile import TileContext
from concourse.vector_clock import ScopedClock
from concourse.masks import make_identity
from concourse import bass_utils

# ----------------------------------------------------------------------------
# problem constants (hardcoded per spec)
B, S, H, E, I_DIM, TOPK = 8, 2048, 768, 8, 1536, 2
T = B * S          # 16384 tokens
N_CORES = 8
TPC = T // N_CORES  # 2048 tokens per core for the router
P = 128
HT = H // P         # 6 h-tiles
IT = I_DIM // P     # 12 i-tiles

F32 = mybir.dt.float32
F32R = mybir.dt.float32r
BF16 = mybir.dt.bfloat16
I32 = mybir.dt.int32

# matmul dtype for the expert MLP: "bf16" | "f32r" | "f32"
MM_DTYPE = os.environ.get("MOE_MM_DTYPE", "f32r")
CHUNK = 512

AUX_LOSS_COEF = 1e-3
Z_LOSS_COEF = 1e-3


# ----------------------------------------------------------------------------
# toolchain workaround: this walrus build only accepts ONE sync-wait per CTRL
# instruction; Tile's end-of-context drain carries one wait per live proc.
# Split them across multiple drains.
def _split_all_waits(nc):
    """Move extra sync-waits (beyond 1 per instruction) onto injected NOPs on
    the same engine, immediately before the instruction.  Semantically
    identical: the engine blocks on each wait in sequence."""
    cur = nc.cur_bb.bb
    for f in nc.m.functions:
        for bb in f.blocks:
            insts = list(bb.instructions)
            out = []
            changed = False
            for inst in insts:
                si = inst.sync_info
                waits = list(si.on_wait) if si and si.on_wait else []
                if len(waits) > 1 and inst.engine in nc.engines:
                    changed = True
                    for w in waits[:-1]:
                        nop = nc.engines[inst.engine].nop(nofuse=True)
                        cur.instructions.pop()
                        nsi = nop.ins.sync_info
                        if nsi is None:
                            nop.ins.sync_info = mybir.SyncInfo(
                                on_wait=[w], on_update=[]
                            )
                        else:
                            nsi.on_wait = [w]
                        out.append(nop.ins)
                    si.on_wait = waits[-1:]
                out.append(inst)
            if changed:
                bb.instructions = out


class _TC(TileContext):
    def _drain_and_barrier(self, tick_clock, wait_clock):
        drain_inst = self.nc.sync.drain()
        wait_clock.add_sem_waits(
            drain_inst.ins, ScopedClock({None: tick_clock.global_clock})
        )
        self.nc.all_engine_barrier()
        popped = self.nc._tile_sem_poison_stack.pop()
        assert popped is self._sem_poison
        self.nc.clear_and_free_semaphores(list(self.sems.allocated().values()))
        self.nc.all_engine_barrier()
        _split_all_waits(self.nc)


def _install_ntff_hook():
    """Make trace=True usable under axon when antenv.axon_hooks is absent."""
    try:
        import antenv  # noqa: F401
    except ImportError:
        return
    if "antenv.axon_hooks" in sys.modules:
        return
    mod = types.ModuleType("antenv.axon_hooks")
    mod._hook = None

    def set_axon_ntff_profile_hook(h):
        mod._hook = h

    def get_axon_ntff_profile_hook():
        return mod._hook

    mod.set_axon_ntff_profile_hook = set_axon_ntff_profile_hook
    mod.get_axon_ntff_profile_hook = get_axon_ntff_profile_hook
    sys.modules["antenv.axon_hooks"] = mod
    import antenv as _a

    _a.axon_hooks = mod
    try:
        from trn_agent_boot.trn_boot import _ntff_profile_via_ctypes

        hook = _ntff_profile_via_ctypes("/opt/axon/libaxon_pjrt.so")
        if hook is not None:
            set_axon_ntff_profile_hook(hook)
    except Exception:
        pass
    # artifact upload isn't available (and isn't wanted) in this container
    bass_utils.upload_artifacts = lambda tmpdir: tmpdir


_install_ntff_hook()


# ----------------------------------------------------------------------------
def build_router_nc():
    """Token-parallel router NEFF.  Inputs: xT [H, TPC] (this core's token
    slice, transposed on host), rw [H, E].  Outputs (device layout,
    token t = tt*128 + p): probs/coef [P, NTT*E], m1/s [P, NTT]."""
    nc = bass.Bass(trn_type="TRN2")
    NTT = TPC // P  # 16 token tiles
    xT = nc.dram_tensor("xT", [H, TPC], F32, kind="ExternalInput")
    rw = nc.dram_tensor("rw", [H, E], F32, kind="ExternalInput")
    probs_o = nc.dram_tensor("probs", [P, NTT * E], F32, kind="ExternalOutput")
    coef_o = nc.dram_tensor("coef", [P, NTT * E], F32, kind="ExternalOutput")
    m1_o = nc.dram_tensor("m1", [P, NTT], F32, kind="ExternalOutput")
    s_o = nc.dram_tensor("s", [P, NTT], F32, kind="ExternalOutput")

    NCH = 4  # input-token chunks for DMA/MM pipelining
    CW = TPC // NCH  # 512

    with _TC(nc) as tc:
        with (
            tc.tile_pool(name="big", bufs=1) as big,
            tc.tile_pool(name="work", bufs=1) as work,
            tc.tile_pool(name="lps", bufs=4, space="PSUM") as lps,
            tc.tile_pool(name="tps", bufs=4, space="PSUM") as tps,
        ):
            ident = work.tile([P, P], F32)
            make_identity(nc, ident[:])
            rw_sb = work.tile([P, HT * E], F32)
            nc.sync.dma_start(
                out=rw_sb[:].rearrange("p (h e) -> p h e", e=E),
                in_=rw.rearrange("(h p) e -> p h e", p=P),
            )
            xT_sb = big.tile([P, HT * TPC], F32)
            xTv = xT_sb[:].rearrange("p (h t) -> p h t", t=TPC)
            for c in range(NCH):
                nc.sync.dma_start(
                    out=xTv[:, :, c * CW : (c + 1) * CW],
                    in_=xT.rearrange("(h p) t -> p h t", p=P)[
                        :, :, c * CW : (c + 1) * CW
                    ],
                )

            # logitsT [8, tok] accumulated over h with rw stationary
            lT_sb = work.tile([8, TPC], F32)
            for c in range(NCH):
                pl = lps.tile([8, CW], F32, tag="lpsum")
                for h in range(HT):
                    nc.tensor.matmul(
                        out=pl[:],
                        lhsT=rw_sb[:, h * E : (h + 1) * E],
                        rhs=xT_sb[:, h * TPC + c * CW : h * TPC + (c + 1) * CW],
                        start=(h == 0),
                        stop=(h == HT - 1),
                    )
                nc.vector.tensor_copy(out=lT_sb[:, c * CW : (c + 1) * CW], in_=pl[:])

            # transpose back to [tok, E] tiles
            logits_sb = work.tile([P, NTT * E], F32)
            for tt in range(NTT):
                pt = tps.tile([P, E], F32, tag="tp")
                nc.tensor.transpose(
                    out=pt[:],
                    in_=lT_sb[:, tt * P : (tt + 1) * P],
                    identity=ident[:8, :8],
                )
                nc.vector.tensor_copy(
                    out=logits_sb[:, tt * E : (tt + 1) * E], in_=pt[:]
                )

            lg = logits_sb[:].rearrange("p (t e) -> p t e", e=E)
            m1 = work.tile([P, NTT], F32)
            nc.vector.reduce_max(out=m1[:], in_=lg, axis=mybir.AxisListType.X)
            m1b = m1[:].to_broadcast([P, NTT, E])

            ex = work.tile([P, NTT * E], F32)
            exg = ex[:].rearrange("p (t e) -> p t e", e=E)
            nc.vector.tensor_sub(out=exg, in0=lg, in1=m1b)
            nc.scalar.activation(
                out=ex[:], in_=ex[:], func=mybir.ActivationFunctionType.Exp
            )

            s = work.tile([P, NTT], F32)
            nc.vector.reduce_sum(out=s[:], in_=exg, axis=mybir.AxisListType.X)
            rs = work.tile([P, NTT], F32)
            nc.vector.reciprocal(out=rs[:], in_=s[:])
            rsb = rs[:].to_broadcast([P, NTT, E])

            probs = work.tile([P, NTT * E], F32)
            pg = probs[:].rearrange("p (t e) -> p t e", e=E)
            nc.vector.tensor_mul(out=pg, in0=exg, in1=rsb)

            # top-2 + renormalized coefficients
            m1p = work.tile([P, NTT], F32)
            nc.vector.reduce_max(out=m1p[:], in_=pg, axis=mybir.AxisListType.X)
            m1pb = m1p[:].to_broadcast([P, NTT, E])

            lt = work.tile([P, NTT * E], F32)
            ltg = lt[:].rearrange("p (t e) -> p t e", e=E)
            nc.vector.tensor_tensor(
                out=ltg, in0=pg, in1=m1pb, op=mybir.AluOpType.is_lt
            )
            p2 = work.tile([P, NTT * E], F32)
            p2g = p2[:].rearrange("p (t e) -> p t e", e=E)
            nc.vector.tensor_mul(out=p2g, in0=pg, in1=ltg)

            m2p = work.tile([P, NTT], F32)
            nc.vector.reduce_max(out=m2p[:], in_=p2g, axis=mybir.AxisListType.X)
            m2pb = m2p[:].to_broadcast([P, NTT, E])

            ge = work.tile([P, NTT * E], F32)
            geg = ge[:].rearrange("p (t e) -> p t e", e=E)
            nc.vector.tensor_tensor(
                out=geg, in0=pg, in1=m2pb, op=mybir.AluOpType.is_ge
            )

            den = work.tile([P, NTT], F32)
            nc.vector.tensor_add(out=den[:], in0=m1p[:], in1=m2p[:])
            rden = work.tile([P, NTT], F32)
            nc.vector.reciprocal(out=rden[:], in_=den[:])
            rdenb = rden[:].to_broadcast([P, NTT, E])

            coef = work.tile([P, NTT * E], F32)
            cg = coef[:].rearrange("p (t e) -> p t e", e=E)
            nc.vector.tensor_mul(out=cg, in0=pg, in1=geg)
            nc.vector.tensor_mul(out=cg, in0=cg, in1=rdenb)

            # outputs in device layout; host reorders
            nc.sync.dma_start(out=probs_o[:, :], in_=probs[:])
            nc.sync.dma_start(out=coef_o[:, :], in_=coef[:])
            nc.sync.dma_start(out=m1_o[:, :], in_=m1[:])
            nc.sync.dma_start(out=s_o[:, :], in_=s[:])
    return nc


def _dev_to_tok(a, ncols=None):
    """[P, NTT*E] device layout -> [TPC, E] token-major (or [P,NTT] -> [TPC])."""
    if ncols is None:
        return np.ascontiguousarray(a.T).reshape(-1)
    p, te = a.shape
    return a.reshape(p, te // ncols, ncols).transpose(1, 0, 2).reshape(-1, ncols)


# ----------------------------------------------------------------------------
def build_expert_nc(C: int, mm_dtype: str):
    """Expert-parallel MLP NEFF for capacity C (multiple of 128).

    Inputs per core: x [T, H] (full), idx [C] int32, w [C] f32,
    wg [H, I], wu [H, I], wd [I, H] (this core's expert).
    Output: yT [H, C] f32 — w-scaled expert output, transposed.
    """
    md = {"bf16": BF16, "f32r": F32R, "f32": F32}[mm_dtype]
    # f32r is bit-identical to f32: declare weight DRAM tensors as f32r so
    # the (fast) HWDGE path loads them without a SWDGE "cast".
    wdt = md if md in (F32, F32R) else F32
    nc = bass.Bass(trn_type="TRN2")
    x = nc.dram_tensor("x", [T, H], F32, kind="ExternalInput")
    idx = nc.dram_tensor("idx", [C, 1], I32, kind="ExternalInput")
    wvec = nc.dram_tensor("w", [C], F32, kind="ExternalInput")
    wg = nc.dram_tensor("wg", [H, I_DIM], wdt, kind="ExternalInput")
    wu = nc.dram_tensor("wu", [H, I_DIM], wdt, kind="ExternalInput")
    wd = nc.dram_tensor("wd", [I_DIM, H], wdt, kind="ExternalInput")
    yT_o = nc.dram_tensor("yT", [H, C], F32, kind="ExternalOutput")

    # chunk widths
    chunks = []
    c0 = 0
    while c0 < C:
        w_ = min(CHUNK, C - c0)
        chunks.append((c0, w_))
        c0 += w_

    with _TC(nc) as tc:
        with (
            tc.tile_pool(name="const", bufs=1) as const,
            tc.tile_pool(name="wpool", bufs=1) as wpool,
            tc.tile_pool(name="xg", bufs=5) as xgp,
            tc.tile_pool(name="xt", bufs=2) as xtp,
            tc.tile_pool(name="hb", bufs=1) as hbp,
            tc.tile_pool(name="sg", bufs=2) as sgp,
            tc.tile_pool(name="yb", bufs=1) as ybp,
            tc.tile_pool(name="tps", bufs=2, space="PSUM") as tps,
            tc.tile_pool(name="gps", bufs=2, space="PSUM") as gps,
            tc.tile_pool(name="ups", bufs=2, space="PSUM") as ups,
            tc.tile_pool(name="yps", bufs=2, space="PSUM") as yps,
        ):
            ident = const.tile([P, P], F32)
            make_identity(nc, ident[:])

            # index / weight vectors
            idx_sb = const.tile([P, C // P], I32)
            nc.sync.dma_start(
                out=idx_sb[:].rearrange("p (a u) -> p a u", u=1),
                in_=idx.rearrange("(a p) u -> p a u", p=P),
            )

            # expert weights, resident in SBUF
            def load_w(dram, n_in_tiles, n_out):
                tl = []
                half = n_out // 2
                for h in range(n_in_tiles):
                    t_ = wpool.tile([P, n_out], md, tag=f"w{dram.name}{h}", name=f"w{dram.name}{h}")
                    tl.append(t_)
                # first column-halves of every tile land first so the first
                # output tiles of the consuming GEMM can start sooner
                for part in range(2):
                    sl = slice(part * half, (part + 1) * half)
                    for h in range(n_in_tiles):
                        if md == dram.dtype:
                            nc.sync.dma_start(
                                out=tl[h][:, sl], in_=dram[h * P : (h + 1) * P, sl]
                            )
                        else:
                            nc.gpsimd.dma_start(
                                out=tl[h][:, sl], in_=dram[h * P : (h + 1) * P, sl]
                            )
                return tl

            wg_sb = load_w(wg, HT, I_DIM)
            wu_sb = load_w(wu, HT, I_DIM)
            wd_sb = load_w(wd, IT, H)

            ones_sb = const.tile([1, P], F32)
            nc.vector.memset(ones_sb[:], 1.0)

            TB = int(os.environ.get("MOE_TB", "3"))

            def emit_gather_transpose(c0, W):
                """Fill one xgT tile [128, (h, W)] for tokens [c0, c0+W)."""
                ntt = W // P
                xt_one = xtp.tile([P, HT * W], md, tag="xt", name=f"xt_{c0}")
                xt_v = xt_one[:].rearrange("p (h w) -> p h w", w=W)
                for j in range(ntt):
                    xg = xgp.tile([P, H], F32, tag="xg", name=f"xg_{c0}_{j}")
                    nc.gpsimd.indirect_dma_start(
                        out=xg[:],
                        out_offset=None,
                        in_=x[:, :],
                        in_offset=bass.IndirectOffsetOnAxis(
                            ap=idx_sb[:, c0 // P + j : c0 // P + j + 1], axis=0
                        ),
                    )
                    # TB transposes per PSUM tile, one batched copy per TB
                    for g in range(HT // TB):
                        pt = tps.tile([P, TB * P], F32, tag="tp", name=f"tp{g}_{c0}_{j}")
                        for k in range(TB):
                            h = TB * g + k
                            nc.tensor.matmul(
                                out=pt[:, k * P : (k + 1) * P],
                                lhsT=xg[:, h * P : (h + 1) * P],
                                rhs=ident[:],
                                is_transpose=True,
                                start=True,
                                stop=True,
                                skip_group_check=True,
                            )
                        nc.vector.tensor_copy(
                            out=xt_v[:, TB * g : TB * g + TB, j * P : (j + 1) * P],
                            in_=pt[:].rearrange("p (k q) -> p k q", q=P),
                        )
                return xt_one

            def emit_gemm_phase(c0, W, xt_one):
                # broadcast w row across partitions via PE
                w_sl = ybp.tile([1, W], F32, tag="wsl", name=f"wsl_{c0}", bufs=2)
                nc.sync.dma_start(out=w_sl[:], in_=wvec[None, c0 : c0 + W])
                wps = tps.tile([P, W], F32, tag="tp", name=f"wps_{c0}")
                nc.tensor.matmul(
                    out=wps[:],
                    lhsT=ones_sb[:],
                    rhs=w_sl[:],
                    start=True,
                    stop=True,
                )
                wbc = ybp.tile([P, W], F32, tag="wbc_sb", name=f"wbc_{c0}", bufs=2)
                nc.vector.tensor_copy(out=wbc[:], in_=wps[:])

                # gate/up + swiglu -> hT tiles [128i, W]
                h_tiles = []
                for i in range(IT):
                    pg_ = gps.tile([P, W], F32, tag="gp")
                    pu_ = ups.tile([P, W], F32, tag="up")
                    for h in range(HT):
                        nc.tensor.matmul(
                            out=pg_[:],
                            lhsT=wg_sb[h][:, i * P : (i + 1) * P],
                            rhs=xt_one[:, h * W : (h + 1) * W],
                            start=(h == 0),
                            stop=(h == HT - 1),
                        )
                    for h in range(HT):
                        nc.tensor.matmul(
                            out=pu_[:],
                            lhsT=wu_sb[h][:, i * P : (i + 1) * P],
                            rhs=xt_one[:, h * W : (h + 1) * W],
                            start=(h == 0),
                            stop=(h == HT - 1),
                        )
                    sg_ = sgp.tile([P, W], F32, tag="sg")
                    nc.scalar.activation(
                        out=sg_[:], in_=pg_[:], func=mybir.ActivationFunctionType.Silu
                    )
                    ht = hbp.tile([P, W], md, tag=f"h{i}", name=f"h{i}_{c0}")
                    nc.vector.tensor_mul(out=ht[:], in0=sg_[:], in1=pu_[:])
                    h_tiles.append(ht)

                # down projection + scale + store
                y_sb = ybp.tile([P, HT * W], F32, tag="y", name=f"y_{c0}")
                for o in range(HT):
                    py = yps.tile([P, W], F32, tag="yp", name=f"yp{o}_{c0}")
                    for i in range(IT):
                        nc.tensor.matmul(
                            out=py[:],
                            lhsT=wd_sb[i][:, o * P : (o + 1) * P],
                            rhs=h_tiles[i][:],
                            start=(i == 0),
                            stop=(i == IT - 1),
                        )
                    nc.vector.tensor_mul(
                        out=y_sb[:, o * W : (o + 1) * W], in0=py[:], in1=wbc[:]
                    )
                nc.sync.dma_start(
                    out=yT_o.rearrange("(o p) c -> p o c", p=P)[:, :, c0 : c0 + W],
                    in_=y_sb[:].rearrange("p (o c) -> p o c", c=W),
                )

            # software pipeline: chunk k+1's gather/transpose is emitted before
            # chunk k's GEMM phase, so its PSUM->SBUF casts overlap GEMM MMs.
            pend = None
            for c0, W in chunks:
                xt_one = emit_gather_transpose(c0, W)
                if pend is not None:
                    emit_gemm_phase(*pend)
                pend = (c0, W, xt_one)
            emit_gemm_phase(*pend)
    return nc


# ----------------------------------------------------------------------------
_NC_CACHE = {}


def _get_router_nc():
    if "router" not in _NC_CACHE:
        _NC_CACHE["router"] = build_router_nc()
    return _NC_CACHE["router"]


def _get_expert_nc(C, mm_dtype):
    key = ("expert", C, mm_dtype)
    if key not in _NC_CACHE:
        _NC_CACHE[key] = build_expert_nc(C, mm_dtype)
    return _NC_CACHE[key]


def _run(nc, in_maps, **kw):
    return bass_utils.run_bass_kernel_spmd(
        nc, in_maps, core_ids=list(range(N_CORES)), **kw
    )


def kernel(hidden_states, router_w, w_gate, w_up, w_down, _profile=None):
    x = np.ascontiguousarray(np.asarray(hidden_states, np.float32)).reshape(T, H)
    rw = np.ascontiguousarray(np.asarray(router_w, np.float32))
    wg = np.ascontiguousarray(np.asarray(w_gate, np.float32))
    wu = np.ascontiguousarray(np.asarray(w_up, np.float32))
    wd = np.ascontiguousarray(np.asarray(w_down, np.float32))

    prof = _profile if _profile is not None else {}

    # ---- NEFF-A: router ----
    xT = np.ascontiguousarray(x.T)  # [H, T]
    nc_a = _get_router_nc()
    in_maps = [
        {"xT": np.ascontiguousarray(xT[:, c * TPC : (c + 1) * TPC]), "rw": rw}
        for c in range(N_CORES)
    ]
    res_a = _run(nc_a, in_maps, **prof.get("a_kw", {}))
    prof["res_a"] = res_a

    probs = np.concatenate([_dev_to_tok(r["probs"], E) for r in res_a.results], 0)
    coef = np.concatenate([_dev_to_tok(r["coef"], E) for r in res_a.results], 0)
    m1 = np.concatenate([_dev_to_tok(r["m1"]) for r in res_a.results], 0)
    sums = np.concatenate([_dev_to_tok(r["s"]) for r in res_a.results], 0)

    # ---- host: routing compaction + losses (exact, fp64) ----
    mask = coef > 0.0
    counts = mask.sum(0)
    C = max(int(math.ceil(counts.max() / P) * P), P)
    idx_arr = np.zeros((N_CORES, C, 1), np.int32)
    w_arr = np.zeros((N_CORES, C), np.float32)
    idx_list = []
    for e in range(E):
        ie = np.nonzero(mask[:, e])[0].astype(np.int32)
        idx_list.append(ie)
        idx_arr[e, : len(ie), 0] = ie
        w_arr[e, : len(ie)] = coef[ie, e]

    Pm = probs.astype(np.float64).mean(0)
    f = mask.astype(np.float64).mean(0) / TOPK
    lb_loss = np.float32(AUX_LOSS_COEF * E * np.sum(f * Pm))
    lse = np.log(sums.astype(np.float64)) + m1.astype(np.float64)
    z_loss = np.float32(Z_LOSS_COEF * np.mean(lse**2))

    # ---- NEFF-B: experts ----
    nc_b = _get_expert_nc(C, MM_DTYPE)
    in_maps = [
        {
            "x": x,
            "idx": idx_arr[e],
            "w": w_arr[e],
            "wg": np.ascontiguousarray(wg[e]),
            "wu": np.ascontiguousarray(wu[e]),
            "wd": np.ascontiguousarray(wd[e]),
        }
        for e in range(E)
    ]
    res_b = _run(nc_b, in_maps, **prof.get("b_kw", {}))
    prof["res_b"] = res_b

    out = np.zeros((T, H), np.float32)
    for e in range(E):
        ie = idx_list[e]
        out[ie] += res_b.results[e]["yT"].T[: len(ie)]

    return out.reshape(B, S, H), lb_loss, z_loss
